# revision 2
# baseline (speedup 1.0000x reference)
"""CosSim2D (3x3, same-pad) Trainium2 kernel, 8-core batch-parallel. v5.

Design (per core = one 224x224x32 image):
  - Host packs the padded image channel-major as ONE flat strip
    [32, 51104] bf16 (226x226 rows flattened, zero tail).  HBM holds
    a single copy; two dy-shifted replicas (+226, +452 px) are made
    on-chip with SBUF->SBUF DMA into partitions 32-63 / 64-95, so a
    K=96 matmul covers taps (0,dx)+(1,dx)+(2,dx) at once.
  - 112 chunks of 452 px, 4 chunk-slots per super-round (28 SRs).
    Per SR: 12 matmuls (3 dx-passes x 4 col-tiles), K=96, PSUM
    accumulation only 3-deep -> half the PSUM write-port traffic of
    the 6-pass v4 layout.
  - Evac: PSUM [128,452] f32 -> bf16 (Vector/Scalar alternating);
    one output DMA per TWO SRs.
  - Norm + power: entirely on host.
"""

import numpy as np

import concourse.bass as bass
import concourse.mybir as mybir
import concourse.tile as tile
from concourse import bacc
from concourse.bass_utils import run_bass_kernel_spmd

K = 3
EPS = 1e-12
H = W = 224
C = 32
F = 32
B = 8
XP = 226                  # padded row stride
LSTRIP = XP * XP          # 51076 valid px
LX = 51104                # padded strip length (mult of 32)
CH = 452                  # px per chunk
NSLOT = 4                 # chunk slots per super-round (PSUM col groups)
NCHUNK = 112              # chunks total (112*452 = 50624 >= 223*226+224)
SR = NCHUNK // NSLOT      # 28 super-rounds

_compiled = None
TRACE = False
LAST_PROFILE = None


def _build():
    nc = bacc.Bacc()
    f32 = mybir.dt.float32
    bf16 = mybir.dt.bfloat16

    xh = nc.declare_dram_parameter("xh", [32, LX], bf16, isOutput=False)
    wt = nc.declare_dram_parameter("wt", [128, 3 * F], bf16, isOutput=False)
    odev = nc.declare_dram_parameter(
        "odev", [SR // 2, 128, 2 * CH], bf16, isOutput=True
    )

    with tile.TileContext(nc) as tc:
        with (
            tc.tile_pool(name="consts", bufs=1) as consts,
            tc.tile_pool(name="xin", bufs=1) as xin_pool,
            tc.tile_pool(name="outp", bufs=6) as out_pool,
            tc.tile_pool(name="psum", bufs=4, space="PSUM") as psum_pool,
        ):
            WT = consts.tile([128, 3 * F], bf16, tag="WT")
            nc.sync.dma_start(out=WT, in_=wt[:, :])

            X = xin_pool.tile([128, LX], bf16, tag="X")
            # HBM pieces land in partitions 0-31; replicas for the dy=1
            # (+226) and dy=2 (+452) taps go to partitions 32-63 / 64-95.
            bounds = [0, 2712]
            while bounds[-1] < LX:
                bounds.append(min(LX, bounds[-1] + 3616))
            for a, b in zip(bounds[:-1], bounds[1:]):
                nc.sync.dma_start(out=X[0:32, a:b], in_=xh[:, a:b])
                a1 = max(a - XP, 0)
                nc.scalar.dma_start(
                    out=X[32:64, a1 : b - XP], in_=X[0:32, a1 + XP : b]
                )
                a2 = max(a - 2 * XP, 0)
                nc.scalar.dma_start(
                    out=X[64:96, a2 : b - 2 * XP], in_=X[0:32, a2 + 2 * XP : b]
                )

            O = None
            for s in range(SR):
                P = psum_pool.tile([128, CH], f32, tag="P", name=f"P_{s}")
                # 3 accumulating passes o (dx offset), 4 col-tiles m.
                for o in range(3):
                    for m in range(NSLOT):
                        base = (NSLOT * s + m) * CH + o
                        nc.tensor.matmul(
                            P[32 * m : 32 * m + 32, :],
                            WT[0:96, 32 * o : 32 * o + 32],
                            X[0:96, base : base + CH],
                            start=(o == 0),
                            stop=(o == 2),
                            tile_position=(0, 32 * m),
                        )
                if s % 2 == 0:
                    O = out_pool.tile(
                        [128, 2 * CH], bf16, tag="O", name=f"O_{s // 2}"
                    )
                dst = O[:, (s % 2) * CH : (s % 2) * CH + CH]
                if s % 2 == 0:
                    nc.vector.tensor_copy(dst, P)
                else:
                    nc.scalar.copy(dst, P)
                    nc.sync.dma_start(out=odev[s // 2], in_=O)

    nc.compile()
    return nc


def _host_pack(image_b):
    """[224,224,32] f32 -> xh [32, LX] bf16: one flat padded strip."""
    import ml_dtypes

    padded = np.zeros((XP, XP, C), dtype=np.float32)
    padded[1:225, 1:225, :] = image_b
    xh = np.zeros((32, LX), dtype=ml_dtypes.bfloat16)
    xh[:, :LSTRIP] = padded.transpose(2, 0, 1).reshape(C, LSTRIP).astype(
        ml_dtypes.bfloat16
    )
    return xh


def _host_weights(w, qtv):
    import ml_dtypes

    w0 = w[0].astype(np.float32)  # [288, 32], row index = t*C + c
    wn = np.sqrt(np.maximum((w0 * w0).sum(axis=0), np.float32(EPS))) + qtv
    wnorm = (w0 / wn[None, :]).astype(np.float32)
    wt9 = wnorm.reshape(3, 3, C, F)  # [dy, dx, c, f]
    # lhsT for pass o: rows 32g..32g+32 = w[dy=g, dx=o] (partition group
    # g holds the strip shifted by g*XP).
    blk = np.zeros((128, 3 * F), dtype=np.float32)
    for g in range(3):
        for o in range(3):
            blk[32 * g : 32 * g + 32, 32 * o : 32 * o + 32] = wt9[g, o]
    return blk.astype(ml_dtypes.bfloat16)


_ILOCAL = None


def _ilocal():
    global _ILOCAL
    if _ILOCAL is None:
        y, x = np.mgrid[0:H, 0:W]
        _ILOCAL = (y * XP + x).reshape(-1)
    return _ILOCAL


def _host_unpack(odev_b):
    """odev [SR/2, 128, 2*CH] bf16 -> conv [H*W, F] f32."""
    arr = np.asarray(odev_b).astype(np.float32)
    arr = arr.reshape(SR // 2, NSLOT, F, 2, CH)   # [sp, m, f, h, c]
    arr = arr.transpose(0, 3, 1, 4, 2)            # [sp, h, m, c, f]
    conv = arr.reshape(NCHUNK * CH, F)            # b = 452*(8sp+4h+m)+c
    return conv[_ilocal(), :]


def kernel(image, w, p, q):
    global _compiled
    image = np.asarray(image)
    w = np.asarray(w, dtype=np.float32)
    p = np.asarray(p, dtype=np.float32)
    q = np.asarray(q, dtype=np.float32)

    qtv = np.float32(np.float32(q[0]) * np.float32(q[0]) / np.float32(10.0))
    wt_full = _host_weights(w, qtv)

    in_maps = []
    for b in range(B):
        in_maps.append(
            {"xh": _host_pack(image[b].astype(np.float32)), "wt": wt_full}
        )

    if _compiled is None:
        _compiled = _build()
    nc = _compiled

    global LAST_PROFILE
    res = run_bass_kernel_spmd(
        nc, in_maps, core_ids=list(range(B)), trace=TRACE
    )
    LAST_PROFILE = res

    e = (p * p) / np.float32(100.0)  # per-filter exponent
    out = np.empty((B, H * W, F), dtype=np.float32)
    pow_is_identity = np.allclose(e, 1.0, rtol=0, atol=0)
    for b in range(B):
        img = image[b].astype(np.float32)
        s2 = (img * img).sum(axis=-1)
        s2p = np.zeros((XP, XP), dtype=np.float32)
        s2p[1:225, 1:225] = s2
        box = np.zeros((H, W), dtype=np.float32)
        for dy in range(K):
            for dx in range(K):
                box += s2p[dy : dy + H, dx : dx + W]
        ns = np.sqrt(np.maximum(box, np.float32(EPS))) + qtv
        inv_ns = (np.float32(1.0) / ns).reshape(H * W, 1)

        sim = _host_unpack(res.results[b]["odev"]) * inv_ns
        if pow_is_identity:
            out[b] = sim
        else:
            out[b] = np.sign(sim) * np.power(
                np.abs(sim) + np.float32(EPS), e[None, :]
            )
    return out.reshape(B, H, W, F)
